# revision 32
# baseline (speedup 1.0000x reference)
"""Multi-head attention (B=2, S=2048, D=1024, H=16) on 8 Trainium2 cores.

Sharding: core c -> (batch b = c//4, head-group g = c%4, 4 heads each).
Tensor-parallel over heads within a batch; the output projection is done
per head-group against the matching Wo column slice and the partial
[S, D] results are summed on the host (plus the folded biases bo + Wo@bv).

All on-device matmuls run in float32r (full-rate PE streaming); exp runs on
the scalar engine in 1024-wide tiles; the softmax denominator comes from a
ones-column appended to V in the PV matmul.
"""

from contextlib import ExitStack

import numpy as np

import concourse.bacc as bacc
import concourse.tile as tile
from concourse import mybir

D_MODEL = 1024
NUM_HEADS = 16
D_K = 64
B = 2
S_FULL = 2048
N_CORES = 8
GH = 4              # heads per core
GJ = GH * D_K       # 256 columns per head-group

F32 = mybir.dt.float32
F32R = mybir.dt.float32r
AF = mybir.ActivationFunctionType
ALU = mybir.AluOpType


def build_nc(S=S_FULL, SB=512):
    """Build + compile the per-core program (identical on all 8 cores)."""
    HB = min(2 * SB, S)   # sq half-width (2 blocks)
    assert S % HB == 0
    NH = S // HB      # number of sq halves
    ST = S // 128     # sk tiles
    DT = D_MODEL // 128
    JT = GJ // 128    # 2 j-tiles (2 heads each)

    nc = bacc.Bacc("TRN2", target_bir_lowering=False, debug=False)

    NCH_ = S // SB
    DT_ = D_MODEL // 128
    xqT = nc.dram_tensor("xqT", [NCH_, 128, DT_, SB], F32R, kind="ExternalInput").ap()
    xkT = nc.dram_tensor("xkT", [NCH_, 128, DT_, SB], F32R, kind="ExternalInput").ap()
    xvT = nc.dram_tensor("xvT", [NCH_, 128, DT_, SB], F32R, kind="ExternalInput").ap()
    wqT = nc.dram_tensor("wqT", [128, DT_, GJ], F32R, kind="ExternalInput").ap()
    wkT = nc.dram_tensor("wkT", [128, DT_, GJ], F32R, kind="ExternalInput").ap()
    wvT = nc.dram_tensor("wvT", [128, DT_, GJ], F32R, kind="ExternalInput").ap()
    woT = nc.dram_tensor("woT", [128, GJ // 128, D_MODEL], F32R, kind="ExternalInput").ap()
    bq = nc.dram_tensor("bq", [128, GJ // 128], F32, kind="ExternalInput").ap()
    bk = nc.dram_tensor("bk", [128, GJ // 128], F32, kind="ExternalInput").ap()
    yT = nc.dram_tensor("yT", [D_MODEL, S], F32, kind="ExternalOutput").ap()

    with tile.TileContext(nc) as tc:
        with ExitStack() as ctx:
            cpool = ctx.enter_context(tc.tile_pool(name="const", bufs=1))
            xs_pool = ctx.enter_context(tc.tile_pool(name="xs", bufs=4))
            p_pool = ctx.enter_context(tc.tile_pool(name="pt", bufs=5))
            y_pool = ctx.enter_context(tc.tile_pool(name="ys", bufs=4))
            s_pool = ctx.enter_context(tc.tile_pool(name="sm", bufs=3))
            ps_s = ctx.enter_context(tc.tile_pool(name="ps2", bufs=2, space="PSUM"))
            ps_o = ctx.enter_context(tc.tile_pool(name="po2", bufs=4, space="PSUM"))

            # ---- persistent SBUF ----
            wq_sb = cpool.tile([128, DT, GJ], F32R, tag="wq")
            wk_sb = cpool.tile([128, DT, GJ], F32R, tag="wk")
            wv_sb = cpool.tile([128, DT, GJ], F32R, tag="wv")
            wo_sb = cpool.tile([128, JT, D_MODEL], F32R, tag="wo")
            bq_sb = cpool.tile([128, JT], F32, tag="bq")
            bk_sb = cpool.tile([128, JT], F32, tag="bk")
            nc.scalar.dma_start(wk_sb[:], wkT)
            nc.scalar.dma_start(wv_sb[:], wvT)
            nc.scalar.dma_start(wq_sb[:], wqT)
            nc.scalar.dma_start(bq_sb[:], bq)
            nc.scalar.dma_start(bk_sb[:], bk)

            qhT_sb = cpool.tile([128, JT, S], F32R, tag="qhT")
            khT_sb = cpool.tile([128, JT, S], F32R, tag="khT")
            vh_sb = cpool.tile([128, ST, GH, 65], F32R, tag="vh")
            oall_sb = cpool.tile([128, JT, S], F32R, tag="oall")

            ones_sb = cpool.tile([128, 1], F32, tag="ones")
            nc.vector.memset(ones_sb[:], 1.0)
            nc.vector.tensor_copy(
                vh_sb[:, :, :, 64:65],
                ones_sb[:, None, :].broadcast_to([128, ST, GH, 1]),
            )

            # ---- stage helpers ----
            def a_chunk(which, sb):
                """Project one 512-wide chunk of x (k/v/q) on the PE."""
                xT, w_sb, b_sb, outT, dma_eng = {
                    "k": (xkT, wk_sb, bk_sb, khT_sb, nc.sync),
                    "v": (xvT, wv_sb, None, None, nc.scalar),
                    "q": (xqT, wq_sb, bq_sb, qhT_sb, nc.sync),
                }[which]
                ss = slice(sb * SB, (sb + 1) * SB)
                xs = xs_pool.tile([128, DT, SB], F32R, tag="xs", name=f"xs_{which}{sb}")
                dma_eng.dma_start(xs[:], xT[sb])
                ps = ps_s.tile([128, JT * SB], F32, tag="ps", name=f"psa_{which}{sb}")
                if outT is not None:
                    # qhT / khT: [GJ, S] transposed projections + bias
                    for jt in range(JT):
                        sl = slice(jt * SB, (jt + 1) * SB)
                        for d in range(DT):
                            nc.tensor.matmul(
                                ps[:, sl],
                                w_sb[:, d, jt * 128:(jt + 1) * 128],
                                xs[:, d, :],
                                start=(d == 0),
                                stop=(d == DT - 1),
                            )
                        nc.vector.tensor_scalar_add(
                            outT[:, jt, ss], ps[:, sl], b_sb[:, jt:jt + 1]
                        )
                else:
                    # vh: normal layout [S, GJ], x tiles stationary
                    for stl in range(SB // 128):
                        st = sb * (SB // 128) + stl
                        sl = slice(stl * GJ, (stl + 1) * GJ)
                        for d in range(DT):
                            nc.tensor.matmul(
                                ps[:, sl],
                                xs[:, d, stl * 128:(stl + 1) * 128],
                                wv_sb[:, d, :],
                                start=(d == 0),
                                stop=(d == DT - 1),
                            )
                        nc.vector.tensor_copy(
                            vh_sb[:, st, :, 0:64],
                            ps[:, sl].rearrange("p (h e) -> p h e", h=GH),
                        )

            # Stage B: the two heads of a pair sit at partition bases 0 / 64
            # of the same j-tile, so their K=64 score matmuls land on disjoint
            # PE row groups and run concurrently; one 1024-wide exp covers both.
            po_live = {}
            po_pend = {}

            def b_pair(hf, hp, st_lo, st_hi, fillers=None):
                hs = slice(hf * SB, (hf + 1) * SB)
                jt = hp
                if st_lo == 0:
                    po_a = ps_o.tile([65, SB], F32, tag="po", name=f"po_{hf}_{hp}_a")
                    po_b = ps_o.tile([65, SB], F32, tag="po", name=f"po_{hf}_{hp}_b")
                    po_live[(hf, hp)] = [po_a, po_b]
                po_ab = po_live[(hf, hp)]

                def pv_mms(st, pt):
                    for hl in range(2):
                        nc.tensor.matmul(
                            po_ab[hl][:],
                            vh_sb[:, st, 2 * hp + hl, :],
                            pt[:, hl * SB:(hl + 1) * SB],
                            start=(st == 0),
                            stop=(st == ST - 1),
                        )

                # scores run one st ahead of PV so the PE never waits on exp
                pend = po_pend.pop((hf, hp), None)
                for st in range(st_lo, st_hi):
                    ps = ps_s.tile([128, JT * SB], F32, tag="ps",
                                   name=f"psb_{hf}_{hp}_{st}")
                    for hl in range(2):
                        base = 64 * hl
                        nc.tensor.matmul(
                            ps[:, hl * SB:(hl + 1) * SB],
                            khT_sb[base:base + 64, jt, st * 128:(st + 1) * 128],
                            qhT_sb[base:base + 64, jt, hs],
                            start=True, stop=True,
                        )
                    if pend is not None:
                        pv_mms(*pend)
                    pt = p_pool.tile([128, JT * SB], F32R, tag="pt",
                                     name=f"pt_{hf}_{hp}_{st}")
                    nc.scalar.activation(pt[:], ps[:], AF.Exp, scale=0.125)
                    pend = (st, pt)
                    if fillers and st >= st_lo + 11:
                        fillers.pop(0)()
                if st_hi == ST:
                    if pend is not None:
                        pv_mms(*pend)
                else:
                    po_pend[(hf, hp)] = pend
                if st_hi == ST:
                    for hl in range(2):
                        base = 64 * hl
                        po = po_ab[hl]
                        rcp = s_pool.tile([1, SB], F32, tag="rcp",
                                          name=f"rcp_{hf}_{hp}_{hl}")
                        nc.vector.reciprocal(rcp[:], po[64:65, :])
                        bcast = s_pool.tile([64, SB], F32, tag="bcast",
                                            name=f"bc_{hf}_{hp}_{hl}")
                        nc.gpsimd.partition_broadcast(bcast[:], rcp[:])
                        nc.vector.tensor_mul(
                            oall_sb[base:base + 64, jt, hs], po[0:64, :], bcast[:]
                        )

            def c_block(hf):
                hs = slice(hf * SB, (hf + 1) * SB)
                yr = yT.rearrange("(t p) s -> t p s", p=128)
                for mt in range(DT):
                    pc = ps_o.tile([128, SB], F32, tag="po", name=f"pc_{hf}_{mt}")
                    for kt in range(JT):
                        nc.tensor.matmul(
                            pc[:],
                            wo_sb[:, kt, mt * 128:(mt + 1) * 128],
                            oall_sb[:, kt, hs],
                            start=(kt == 0),
                            stop=(kt == JT - 1),
                        )
                    yt = y_pool.tile([128, SB], F32, tag="yt", name=f"yt_{hf}_{mt}")
                    nc.vector.tensor_copy(yt[:], pc[:])
                    (nc.sync if mt % 2 else nc.scalar).dma_start(
                        yr[mt, :, hs], yt[:]
                    )

            yr = yT.rearrange("(t p) s -> t p s", p=128)

            def c_units(hf):
                hs = slice(hf * SB, (hf + 1) * SB)
                units = []
                for mt in range(DT):
                    def u(mt=mt):
                        pc = ps_o.tile([128, SB], F32, tag="po",
                                       name=f"pc_{hf}_{mt}")
                        for kt in range(JT):
                            nc.tensor.matmul(
                                pc[:],
                                wo_sb[:, kt, mt * 128:(mt + 1) * 128],
                                oall_sb[:, kt, hs],
                                start=(kt == 0),
                                stop=(kt == JT - 1),
                            )
                        yt = y_pool.tile([128, SB], F32, tag="yt",
                                         name=f"yt_{hf}_{mt}")
                        nc.vector.tensor_copy(yt[:], pc[:])
                        (nc.sync if mt % 2 else nc.scalar).dma_start(
                            yr[mt, :, hs], yt[:]
                        )
                    units.append(u)
                return units

            def q_load(sb):
                xs = xs_pool.tile([128, DT, SB], F32R, tag="xs",
                                  name=f"xs_qf{sb}")
                nc.sync.dma_start(xs[:], xqT[sb])
                return xs

            def q_proj_units(sb, xs):
                ss = slice(sb * SB, (sb + 1) * SB)
                units = []
                for jt in range(JT):
                    def u(jt=jt, xs=xs):
                        psq = ps_o.tile([128, SB], F32, tag="po",
                                        name=f"psq_{sb}_{jt}")
                        for d in range(DT):
                            nc.tensor.matmul(
                                psq[:],
                                wq_sb[:, d, jt * 128:(jt + 1) * 128],
                                xs[:, d, :],
                                start=(d == 0),
                                stop=(d == DT - 1),
                            )
                        nc.vector.tensor_scalar_add(
                            qhT_sb[:, jt, ss], psq[:], bq_sb[:, jt:jt + 1]
                        )
                    units.append(u)
                return units

            # ---- fused schedule ----
            # hf block 0 of stage B is interleaved with stage A chunk-wise:
            # B consumes k/v sk-tiles as each chunk's projection lands.
            NCH = S // SB           # chunks
            STB = ST // NCH         # sk-tiles per chunk
            a_chunk("k", 0)
            a_chunk("v", 0)
            a_chunk("q", 0)
            q_next = None
            for sb in range(NCH):
                if sb > 0:
                    a_chunk("k", sb)
                    a_chunk("v", sb)
                if sb == NCH - 1 and NCH > 1:
                    q_next = q_load(1)
                for hp in range(GH // 2):
                    b_pair(0, hp, sb * STB, (sb + 1) * STB)
            nc.scalar.dma_start(wo_sb[:], woT)
            for hf in range(1, NCH):
                qu = q_proj_units(hf, q_next)
                qu[0]()
                fillers = [qu[1]] + c_units(hf - 1)
                if hf + 1 < NCH:
                    q_next = q_load(hf + 1)
                b_pair(hf, 0, 0, ST, fillers)
                b_pair(hf, 1, 0, ST, fillers)
                for u in fillers:
                    u()
            c_block(NCH - 1)
            if NCH == 1:
                c_block(0)

    nc.compile()
    return nc


_NC_CACHE = {}


def _get_nc(S=S_FULL):
    if S not in _NC_CACHE:
        _NC_CACHE[S] = build_nc(S)
    return _NC_CACHE[S]


def make_in_maps(q, k, v, Wq, bq, Wk, bk, Wv, bv, Wo, bo, S=S_FULL):
    q = np.asarray(q, np.float32)
    k = np.asarray(k, np.float32)
    v = np.asarray(v, np.float32)
    Wq = np.asarray(Wq, np.float32)
    Wk = np.asarray(Wk, np.float32)
    Wv = np.asarray(Wv, np.float32)
    Wo = np.asarray(Wo, np.float32)
    bq = np.asarray(bq, np.float32)
    bk = np.asarray(bk, np.float32)

    SB = 512
    NCH = S // SB
    DT = D_MODEL // 128

    def xtile(x):
        # [S, D] -> xT [D, S] -> [NCH, 128, DT, SB]: t[sb, p, d, s] = x[sb*SB+s, d*128+p]
        xT = x.T  # [D, S]
        return np.ascontiguousarray(
            xT.reshape(DT, 128, NCH, SB).transpose(2, 1, 0, 3)
        )

    def wtile(wT):
        # [D, GJ] -> [128, DT, GJ]
        return np.ascontiguousarray(wT.reshape(DT, 128, GJ).transpose(1, 0, 2))

    in_maps = []
    for c in range(N_CORES):
        b, g = divmod(c, GH)
        sl = slice(g * GJ, (g + 1) * GJ)
        woT = Wo[:, sl].T  # [GJ, D]
        in_maps.append({
            "xqT": xtile(q[b, :S]),
            "xkT": xtile(k[b, :S]),
            "xvT": xtile(v[b, :S]),
            "wqT": wtile(Wq[sl].T),
            "wkT": wtile(Wk[sl].T),
            "wvT": wtile(Wv[sl].T),
            "woT": np.ascontiguousarray(
                woT.reshape(2, 128, D_MODEL).transpose(1, 0, 2)
            ),
            "bq": np.ascontiguousarray(bq[sl].reshape(2, 128).T),
            "bk": np.ascontiguousarray(bk[sl].reshape(2, 128).T),
        })
    return in_maps


def gather_out(results, Wo, bv, bo, S=S_FULL):
    Wo = np.asarray(Wo, np.float32)
    bv = np.asarray(bv, np.float32)
    bo = np.asarray(bo, np.float32)
    out = np.zeros((B, S, D_MODEL), np.float32)
    for c in range(N_CORES):
        out[c // GH] += results[c]["yT"].T
    out += bo + Wo @ bv
    return out


def kernel(q, k, v, Wq, bq, Wk, bk, Wv, bv, Wo, bo):
    from concourse.bass_utils import run_bass_kernel_spmd

    nc = _get_nc(S_FULL)
    in_maps = make_in_maps(q, k, v, Wq, bq, Wk, bk, Wv, bv, Wo, bo)
    res = run_bass_kernel_spmd(nc, in_maps, core_ids=list(range(N_CORES)))
    return gather_out(res.results, Wo, bv, bo)



# revision 33
# speedup vs baseline: 1.2085x; 1.2085x over previous
"""Multi-head attention (B=2, S=2048, D=1024, H=16) on 8 Trainium2 cores.

Sharding: core c -> (batch b = c//4, head-group g = c%4, 4 heads each).
Tensor-parallel over heads within a batch; the output projection is done
per head-group against the matching Wo column slice and the partial
[S, D] results are summed on the host (plus the folded biases bo + Wo@bv).

v3: PV matmuls run P-stationary (exp-scores bf16) with V bf16 as the
65-wide moving operand (64 dims + a ones column accumulating the softmax
denominator). Projections run fully in bf16 (halves the startup DMA);
scores and the output projection stay f32r<->f32r. Softmax normalization
is a batched per-partition reciprocal + broadcast multiply on DVE. The
normalized context is transposed back with PE identity-matmuls. Each
pair's last PV flushes + normalize are carried into the next pair's
first iterations so the Activation engine never drains at boundaries.
All DMAs issue from the SP / Pool queues.
"""

from contextlib import ExitStack

import numpy as np

import concourse.bacc as bacc
import concourse.tile as tile
from concourse import mybir

D_MODEL = 1024
NUM_HEADS = 16
D_K = 64
B = 2
S_FULL = 2048
N_CORES = 8
GH = 4              # heads per core
GJ = GH * D_K       # 256 columns per head-group

F32 = mybir.dt.float32
F32R = mybir.dt.float32r
BF16 = mybir.dt.bfloat16
AF = mybir.ActivationFunctionType
ALU = mybir.AluOpType


def build_nc(S=S_FULL, SB=512):
    """Build + compile the per-core program (identical on all 8 cores)."""
    NCH = S // SB     # chunks (and hf blocks)
    ST = S // 128     # sk tiles
    DT = D_MODEL // 128
    JT = GJ // 128    # 2 j-tiles (2 heads each)
    STB = ST // NCH   # sk-tiles per chunk
    SQT = SB // 128   # sq 128-tiles per hf block

    nc = bacc.Bacc("TRN2", target_bir_lowering=False, debug=False)

    xqT = nc.dram_tensor("xqT", [NCH, 128, DT, SB], BF16, kind="ExternalInput").ap()
    xkT = nc.dram_tensor("xkT", [NCH, 128, DT, SB], BF16, kind="ExternalInput").ap()
    xvT = nc.dram_tensor("xvT", [NCH, 128, DT, SB], BF16, kind="ExternalInput").ap()
    wqT = nc.dram_tensor("wqT", [128, DT, GJ], BF16, kind="ExternalInput").ap()
    wkT = nc.dram_tensor("wkT", [128, DT, GJ], BF16, kind="ExternalInput").ap()
    wvT = nc.dram_tensor("wvT", [128, DT, GJ], BF16, kind="ExternalInput").ap()
    woT = nc.dram_tensor("woT", [128, GJ // 128, D_MODEL], F32R, kind="ExternalInput").ap()
    bq = nc.dram_tensor("bq", [128, GJ // 128], F32, kind="ExternalInput").ap()
    bk = nc.dram_tensor("bk", [128, GJ // 128], F32, kind="ExternalInput").ap()
    ident = nc.dram_tensor("ident", [128, 128], F32, kind="ExternalInput").ap()
    yT = nc.dram_tensor("yT", [D_MODEL, S], BF16, kind="ExternalOutput").ap()

    with tile.TileContext(nc) as tc:
        with ExitStack() as ctx:
            cpool = ctx.enter_context(tc.tile_pool(name="const", bufs=1))
            xk_pool = ctx.enter_context(tc.tile_pool(name="xk", bufs=3))
            xq_pool = ctx.enter_context(tc.tile_pool(name="xq", bufs=2))
            xv_pool = ctx.enter_context(tc.tile_pool(name="xv", bufs=2))
            p_pool = ctx.enter_context(tc.tile_pool(name="pt", bufs=9))
            y_pool = ctx.enter_context(tc.tile_pool(name="ys", bufs=4))
            s_pool = ctx.enter_context(tc.tile_pool(name="sm", bufs=4))
            ps_s = ctx.enter_context(tc.tile_pool(name="ps2", bufs=2, space="PSUM"))
            ps_b = ctx.enter_context(tc.tile_pool(name="pb2", bufs=1, space="PSUM"))

            # Four manually-scheduled 1-bank PSUM slots. The live pair's PV
            # accumulators hold one pair of slots ((A,B) or (C,D),
            # alternating per pair); transient tiles (transpose, out-proj,
            # q-proj) cycle on the opposite pair of slots.
            _pb_state = {"t": ("pbC", "pbD"), "i": 0}

            def pb_tile(name, tag):
                return ps_b.tile([128, 512], F32, tag=tag, name=name, bufs=1)

            def pb_next(name):
                tags = _pb_state["t"]
                tag = tags[_pb_state["i"] % len(tags)]
                _pb_state["i"] += 1
                return pb_tile(name, tag)

            # ---- persistent SBUF ----
            wq_sb = cpool.tile([128, DT, GJ], BF16, tag="wq")
            wk_sb = cpool.tile([128, DT, GJ], BF16, tag="wk")
            wv_sb = cpool.tile([128, DT, GJ], BF16, tag="wv")
            wo_sb = cpool.tile([128, JT, D_MODEL], F32R, tag="wo")
            bq_sb = cpool.tile([128, JT], F32, tag="bq")
            bk_sb = cpool.tile([128, JT], F32, tag="bk")
            id_sb = cpool.tile([128, 128], F32, tag="ident")
            warm_sb = cpool.tile([128, 512], F32R, tag="warm")

            qhT_sb = cpool.tile([128, JT, S], F32R, tag="qhT")
            khT_sb = cpool.tile([128, JT, S], F32R, tag="khT")
            vh_sb = cpool.tile([128, ST, GH, 65], BF16, tag="vh")
            oall_sb = cpool.tile([128, ST, GJ], F32, tag="oall")
            oallT_sb = cpool.tile([128, JT, S], F32R, tag="oallT")

            ones_sb = cpool.tile([128, 1], F32, tag="ones")

            nc.vector.memset(warm_sb[:].bitcast(F32), 0.0)
            nc.vector.memset(ones_sb[:], 1.0)
            nc.vector.tensor_copy(
                vh_sb[:, :, :, 64:65],
                ones_sb[:, None, :].broadcast_to([128, ST, GH, 1]),
            )

            # ---- initial DMAs: x chunks on SP, weights on Pool ----
            xs_k0 = xk_pool.tile([128, DT, SB], BF16, tag="xk", name="xs_k0")
            nc.sync.dma_start(xs_k0[:, 0:4, :], xkT[0][:, 0:4, :])
            nc.sync.dma_start(wk_sb[:], wkT)
            nc.sync.dma_start(xs_k0[:, 4:8, :], xkT[0][:, 4:8, :])
            nc.gpsimd.dma_start(bk_sb[:], bk)
            nc.gpsimd.dma_start(bq_sb[:], bq)
            xs_q0 = xq_pool.tile([128, DT, SB], BF16, tag="xq", name="xs_q0")
            nc.sync.dma_start(xs_q0[:, 0:4, :], xqT[0][:, 0:4, :])
            nc.sync.dma_start(wq_sb[:], wqT)
            nc.sync.dma_start(xs_q0[:, 4:8, :], xqT[0][:, 4:8, :])
            nc.gpsimd.dma_start(id_sb[:], ident)
            nc.gpsimd.dma_start(wv_sb[:], wvT)
            xs_v0 = xv_pool.tile([128, DT, SB], BF16, tag="xv", name="xs_v0")
            nc.sync.dma_start(xs_v0[:], xvT[0])
            # prefetch chunk 1 right behind chunk 0
            xs_k1 = xk_pool.tile([128, DT, SB], BF16, tag="xk", name="xs_k1")
            nc.sync.dma_start(xs_k1[:], xkT[1])
            xs_v1 = xv_pool.tile([128, DT, SB], BF16, tag="xv", name="xs_v1")
            nc.gpsimd.dma_start(xs_v1[:], xvT[1])

            # warmup: keep the PE busy (and p-state ramped) during the
            # initial DMA window; results are never read
            for i in range(10):
                wps = pb_tile(f"warm{i}", ("pbA", "pbB", "pbC", "pbD")[i % 4])
                nc.tensor.matmul(
                    wps[:], warm_sb[:, 0:128], warm_sb[:],
                    start=True, stop=True,
                )
            for i in range(8):
                wps = pb_tile(f"warms{i}", ("pbA", "pbB", "pbC", "pbD")[i % 4])
                nc.tensor.matmul(
                    wps[:, 0:128], warm_sb[:, 0:128], warm_sb[:, 0:128],
                    start=True, stop=True,
                )

            # ---- projections ----
            def kq_proj(which, sb, xs, jts=(0, 1)):
                """Transposed projection chunk -> qhT/khT[:, jt, sb*SB:...]
                through a [128, 1024] ps_s scratch tile (pre-B / window
                boundaries only)."""
                w_sb, b_sb, outT = {
                    "k": (wk_sb, bk_sb, khT_sb),
                    "q": (wq_sb, bq_sb, qhT_sb),
                }[which]
                ss = slice(sb * SB, (sb + 1) * SB)
                ps = ps_s.tile([128, JT * SB], F32, tag="ps",
                               name=f"ps_{which}{sb}")
                for jt in jts:
                    sl = slice(jt * SB, (jt + 1) * SB)
                    for d in range(DT):
                        nc.tensor.matmul(
                            ps[:, sl],
                            w_sb[:, d, jt * 128:(jt + 1) * 128],
                            xs[:, d, :],
                            start=(d == 0),
                            stop=(d == DT - 1),
                        )
                    nc.vector.tensor_scalar_add(
                        outT[:, jt, ss], ps[:, sl], b_sb[:, jt:jt + 1]
                    )

            def v_proj(sb, xs, stls=(0, 1, 2, 3)):
                """Normal-layout projection chunk -> vh (bf16, ones kept)."""
                ps = ps_s.tile([128, JT * SB], F32, tag="ps", name=f"ps_v{sb}")
                for stl in stls:
                    st = sb * (SB // 128) + stl
                    sl = slice(stl * GJ, (stl + 1) * GJ)
                    for d in range(DT):
                        nc.tensor.matmul(
                            ps[:, sl],
                            xs[:, d, stl * 128:(stl + 1) * 128],
                            wv_sb[:, d, :],
                            start=(d == 0),
                            stop=(d == DT - 1),
                        )
                    nc.vector.tensor_copy(
                        vh_sb[:, st, :, 0:64],
                        ps[:, sl].rearrange("p (h e) -> p h e", h=GH),
                    )

            def q_jt_units(sb, xs, jt):
                """q-projection of one j-tile as 4 filler units (2 matmuls
                each) accumulating into a transient pb slot."""
                cell = {}
                units = []
                for g in range(4):
                    def u(g=g):
                        if g == 0:
                            cell["t"] = pb_next(f"qp_{sb}_{jt}")
                        t = cell["t"]
                        for d in (2 * g, 2 * g + 1):
                            nc.tensor.matmul(
                                t[:],
                                wq_sb[:, d, jt * 128:(jt + 1) * 128],
                                xs[:, d, :],
                                start=(d == 0),
                                stop=(d == DT - 1),
                            )
                        if g == 3:
                            nc.vector.tensor_scalar_add(
                                qhT_sb[:, jt, sb * SB:(sb + 1) * SB],
                                t[:], bq_sb[:, jt:jt + 1]
                            )
                    units.append(u)
                return units

            # ---- stage B: scores -> exp -> PV (P-stationary) ----
            acc_live = {}
            pend_live = {}

            def pv_mms(hf, hp, st, pt):
                accs = acc_live[(hf, hp)]
                for hl in range(2):
                    acc = accs[hl]
                    for c in range(SQT):
                        nc.tensor.matmul(
                            acc[:, c, 0:65],
                            pt[:, hl * SB + c * 128:hl * SB + (c + 1) * 128],
                            vh_sb[:, st, 2 * hp + hl, :],
                            start=False,
                            stop=(st == ST - 1),
                            skip_group_check=True,
                        )

            def finish_units(hf, hp):
                """Last PV flushes + softmax normalize of a finished pair,
                as schedulable units (run inside the NEXT pair's loop)."""
                units = []
                for st, pt in pend_live.pop((hf, hp)):
                    units.append(
                        lambda st=st, pt=pt: pv_mms(hf, hp, st, pt))

                def norm():
                    accs = acc_live.pop((hf, hp))
                    for hl in range(2):
                        rcp = s_pool.tile([128, SQT], F32, tag="rcp",
                                          name=f"rcp_{hf}_{hp}_{hl}")
                        nc.vector.reciprocal(
                            rcp[:],
                            accs[hl][:, :, 64:65].rearrange("p c e -> p (c e)"),
                        )
                        nc.vector.tensor_mul(
                            oall_sb[:, hf * SQT:(hf + 1) * SQT,
                                    (2 * hp + hl) * 64:(2 * hp + hl + 1) * 64],
                            accs[hl][:, :, 0:64],
                            rcp[:, :, None].broadcast_to([128, SQT, 64]),
                        )
                units.append(norm)
                return units

            def b_pair(hf, hp, st_lo, st_hi, fillers=None, acc_tags=None,
                       carry=None, lag=2):
                hs = slice(hf * SB, (hf + 1) * SB)
                jt = hp
                if carry:
                    # first own-PV must follow the carried normalize of the
                    # pair whose accumulator slots we reclaim
                    lag = max(lag, len(carry))

                def ensure_accs():
                    if (hf, hp) in acc_live:
                        return
                    accs = []
                    for hl in range(2):
                        t = pb_tile(f"acc_{hf}_{hp}_{hl}", acc_tags[hl])
                        # PSUM only resets via a start=True matmul covering
                        # the bank (DVE memset does not clear PSUM on HW)
                        nc.tensor.matmul(
                            t[:], warm_sb[:, 0:128], warm_sb[:],
                            start=True, stop=True,
                        )
                        accs.append(t.rearrange("p (c e) -> p c e", c=SQT))
                    acc_live[(hf, hp)] = accs
                # PV runs two iterations behind scores so its exp dependency
                # is always satisfied by the time the PE reaches it
                pend = pend_live.pop((hf, hp), [])
                started = (hf, hp) in acc_live
                for st in range(st_lo, st_hi):
                    ps = ps_s.tile([128, JT * SB], F32, tag="ps",
                                   name=f"psb_{hf}_{hp}_{st}")
                    for hl in range(2):
                        base = 64 * hl
                        nc.tensor.matmul(
                            ps[:, hl * SB:(hl + 1) * SB],
                            khT_sb[base:base + 64, jt, st * 128:(st + 1) * 128],
                            qhT_sb[base:base + 64, jt, hs],
                            start=True, stop=True,
                        )
                    if carry:
                        carry.pop(0)()
                    elif fillers:
                        fillers.pop(0)()
                    if started or len(pend) >= lag:
                        started = True
                        take = 2 if len(pend) > 2 else (
                            1 if len(pend) >= 2 else 0)
                        for _ in range(take):
                            ensure_accs()
                            pv_mms(hf, hp, *pend.pop(0))
                    pt = p_pool.tile([128, JT * SB], BF16, tag="pt",
                                     name=f"pt_{hf}_{hp}_{st}")
                    nc.scalar.activation(pt[:], ps[:], AF.Exp, scale=0.125)
                    pend.append((st, pt))
                pend_live[(hf, hp)] = pend

            # ---- stage C: transpose + output projection ----
            yr = yT.rearrange("(t p) s -> t p s", p=128)

            def t_units(hf):
                """Transpose oall[sq, gj] -> oallT[gj, sq] for one hf block."""
                units = []
                for jt2 in range(JT):
                    def u(jt2=jt2):
                        tp = pb_next(f"tp_{hf}_{jt2}")
                        for c in range(SQT):
                            nc.tensor.transpose(
                                tp[:, c * 128:(c + 1) * 128],
                                oall_sb[:, hf * SQT + c,
                                        jt2 * 128:(jt2 + 1) * 128],
                                id_sb[:],
                            )
                        nc.vector.tensor_copy(
                            oallT_sb[:, jt2, hf * SB:(hf + 1) * SB], tp[:]
                        )
                    units.append(u)
                return units

            def c_units(hf, tail=False):
                hs = slice(hf * SB, (hf + 1) * SB)
                units = []
                for mt in range(DT):
                    def u(mt=mt):
                        pc = pb_next(f"pc_{hf}_{mt}")
                        for kt in range(JT):
                            nc.tensor.matmul(
                                pc[:],
                                wo_sb[:, kt, mt * 128:(mt + 1) * 128],
                                oallT_sb[:, kt, hs],
                                start=(kt == 0),
                                stop=(kt == JT - 1),
                            )
                        yt = y_pool.tile([128, SB], BF16, tag="yt",
                                         name=f"yt_{hf}_{mt}")
                        if tail:
                            # Act engine + queue are idle at the tail
                            if mt % 2:
                                nc.vector.tensor_copy(yt[:], pc[:])
                            else:
                                nc.scalar.activation(yt[:], pc[:], AF.Copy)
                            (nc.sync if mt % 2 else nc.scalar).dma_start(
                                yr[mt, :, hs], yt[:]
                            )
                        else:
                            nc.vector.tensor_copy(yt[:], pc[:])
                            (nc.sync if mt % 2 else nc.gpsimd).dma_start(
                                yr[mt, :, hs], yt[:]
                            )
                    units.append(u)
                return units

            def q_load(sb, eng):
                xs = xq_pool.tile([128, DT, SB], BF16, tag="xq",
                                  name=f"xs_qf{sb}")
                eng.dma_start(xs[:], xqT[sb])
                return xs

            # ---- fused schedule ----
            # Pre-B: chunk-0 k/q projections only -- v_proj(0) runs as a
            # filler inside pair (0,1)'s first window so the first scores
            # (and exps) never wait on the later xv0 DMA.
            kq_proj("k", 0, xs_k0)
            kq_proj("q", 0, xs_q0)

            # hf0: both pairs interleaved chunk-wise (PV accumulators occupy
            # all four pb slots); chunk sb+1's k/v projections issue in small
            # bursts at window boundaries so the Act backlog covers them.
            xs_k = {0: xs_k0, 1: xs_k1}
            xs_v = {0: xs_v0, 1: xs_v1}
            q_next = None
            for sb in range(NCH):
                if sb + 2 < NCH:
                    xs_k[sb + 2] = xk_pool.tile([128, DT, SB], BF16, tag="xk",
                                                name=f"xs_k{sb + 2}")
                    nc.sync.dma_start(xs_k[sb + 2][:], xkT[sb + 2])
                    xs_v[sb + 2] = xv_pool.tile([128, DT, SB], BF16, tag="xv",
                                                name=f"xs_v{sb + 2}")
                    nc.gpsimd.dma_start(xs_v[sb + 2][:], xvT[sb + 2])
                if sb + 1 < NCH:
                    bursts = [
                        lambda: kq_proj("k", sb + 1, xs_k[sb + 1], jts=(0,)),
                        lambda: kq_proj("k", sb + 1, xs_k[sb + 1], jts=(1,)),
                        lambda: v_proj(sb + 1, xs_v[sb + 1], stls=(0, 1)),
                        lambda: v_proj(sb + 1, xs_v[sb + 1], stls=(2, 3)),
                    ]
                else:
                    bursts = []
                if sb == 1:
                    nc.gpsimd.dma_start(wo_sb[:], woT)
                if sb == 2:
                    q_next = q_load(1, nc.sync)
                if sb == 3:
                    bursts = [
                        lambda: kq_proj("q", 1, q_next, jts=(0,)),
                        lambda: kq_proj("q", 1, q_next, jts=(1,)),
                    ]
                if sb == 0:
                    # no PVs in pair0's first window (v not projected yet)
                    b_pair(0, 0, 0, STB, acc_tags=("pbA", "pbB"), lag=99)
                    vf = [lambda: v_proj(0, xs_v0, stls=(0, 1)),
                          lambda: v_proj(0, xs_v0, stls=(2, 3))]
                    b_pair(0, 1, 0, STB, fillers=vf,
                           acc_tags=("pbC", "pbD"))
                    for bu in vf:
                        bu()
                    for bu in bursts:
                        bu()
                    continue
                b_pair(0, 0, sb * STB, (sb + 1) * STB,
                       acc_tags=("pbA", "pbB"))
                for bu in bursts[:2]:
                    bu()
                b_pair(0, 1, sb * STB, (sb + 1) * STB,
                       acc_tags=("pbC", "pbD"))
                for bu in bursts[2:4]:
                    bu()

            # hf1..3: the live pair's accumulators alternate (A,B)/(C,D);
            # transients cycle on the opposite slots. Each pair's tail (PV
            # flush + normalize) is carried into the next pair's loop.
            carry = finish_units(0, 0) + finish_units(0, 1)
            tag_flip = {("pbA", "pbB"): ("pbC", "pbD"),
                        ("pbC", "pbD"): ("pbA", "pbB")}
            cur_tags = ("pbC", "pbD")
            for hf in range(1, NCH):
                fillers = []
                if hf + 1 < NCH:
                    q_next = q_load(hf + 1, nc.gpsimd if hf % 2 else nc.sync)
                    fillers += q_jt_units(hf + 1, q_next, 0)
                    fillers += q_jt_units(hf + 1, q_next, 1)
                fillers += t_units(hf - 1) + c_units(hf - 1)
                if hf == NCH - 1:
                    tp_last = t_units(hf)
                for hp in range(2):
                    _pb_state["t"] = cur_tags  # transients on the free pair
                    use = tag_flip[cur_tags]
                    if hf == NCH - 1 and hp == 1:
                        fillers.append(tp_last[0])
                    b_pair(hf, hp, 0, ST, fillers, acc_tags=use, carry=carry)
                    carry = finish_units(hf, hp)
                    cur_tags = use
                for u in fillers:
                    u()
            for u in carry:
                u()
            _pb_state["t"] = ("pbA", "pbB", "pbC", "pbD")
            _pb_state["i"] = 0
            tp_last[1]()
            for u in c_units(NCH - 1, tail=True):
                u()

    nc.compile()
    return nc


_NC_CACHE = {}


def _get_nc(S=S_FULL):
    if S not in _NC_CACHE:
        _NC_CACHE[S] = build_nc(S)
    return _NC_CACHE[S]


def make_in_maps(q, k, v, Wq, bq, Wk, bk, Wv, bv, Wo, bo, S=S_FULL):
    import ml_dtypes

    q = np.asarray(q, np.float32)
    k = np.asarray(k, np.float32)
    v = np.asarray(v, np.float32)
    Wq = np.asarray(Wq, np.float32)
    Wk = np.asarray(Wk, np.float32)
    Wv = np.asarray(Wv, np.float32)
    Wo = np.asarray(Wo, np.float32)
    bq = np.asarray(bq, np.float32)
    bk = np.asarray(bk, np.float32)

    SB = 512
    NCH = S // SB
    DT = D_MODEL // 128
    bf16 = ml_dtypes.bfloat16
    eye = np.ascontiguousarray(np.eye(128, dtype=np.float32))

    def xtile(x):
        # [S, D] -> xT [D, S] -> [NCH, 128, DT, SB]: t[sb, p, d, s] = x[sb*SB+s, d*128+p]
        xT = x.T  # [D, S]
        return np.ascontiguousarray(
            xT.reshape(DT, 128, NCH, SB).transpose(2, 1, 0, 3).astype(bf16)
        )

    def wtile(wT):
        # [D, GJ] -> [128, DT, GJ]
        return np.ascontiguousarray(
            wT.reshape(DT, 128, GJ).transpose(1, 0, 2).astype(bf16))

    in_maps = []
    for c in range(N_CORES):
        b, g = divmod(c, GH)
        sl = slice(g * GJ, (g + 1) * GJ)
        woT = Wo[:, sl].T  # [GJ, D]
        in_maps.append({
            "xqT": xtile(q[b, :S]),
            "xkT": xtile(k[b, :S]),
            "xvT": xtile(v[b, :S]),
            "wqT": wtile(Wq[sl].T),
            "wkT": wtile(Wk[sl].T),
            "wvT": wtile(Wv[sl].T),
            "woT": np.ascontiguousarray(
                woT.reshape(2, 128, D_MODEL).transpose(1, 0, 2)
            ),
            "bq": np.ascontiguousarray(bq[sl].reshape(2, 128).T),
            "bk": np.ascontiguousarray(bk[sl].reshape(2, 128).T),
            "ident": eye,
        })
    return in_maps


def gather_out(results, Wo, bv, bo, S=S_FULL):
    Wo = np.asarray(Wo, np.float32)
    bv = np.asarray(bv, np.float32)
    bo = np.asarray(bo, np.float32)
    out = np.zeros((B, S, D_MODEL), np.float32)
    for c in range(N_CORES):
        out[c // GH] += results[c]["yT"].astype(np.float32).T
    out += bo + Wo @ bv
    return out


def kernel(q, k, v, Wq, bq, Wk, bk, Wv, bv, Wo, bo):
    from concourse.bass_utils import run_bass_kernel_spmd

    nc = _get_nc(S_FULL)
    in_maps = make_in_maps(q, k, v, Wq, bq, Wk, bk, Wv, bv, Wo, bo)
    res = run_bass_kernel_spmd(nc, in_maps, core_ids=list(range(N_CORES)))
    return gather_out(res.results, Wo, bv, bo)
